# revision 1
# baseline (speedup 1.0000x reference)
"""ConVIRT loss kernel for 8 Trainium2 NeuronCores.

Computation (reference):
    vn = v / max(||v||, eps);  un = u / max(||u||, eps)          [8192, 768]
    sim = vn @ un.T / TAU                                        [8192, 8192]
    loss_it = logsumexp(sim, axis=1) - diag(sim)
    loss_ti = logsumexp(sim, axis=0) - diag(sim)
    out = mean(0.75 * loss_it + 0.25 * loss_ti)                  scalar

Sharding: rows of v are split across the 8 cores (1024 rows each); every
core holds all of u.  Core c computes its [1024, 8192] slab of
exp(sim / TAU) on the fly and reduces it two ways:
  - row sums   (free-axis accumulation attached to the exp activation)
  - column sums (partition-axis reduction via a ones-vector matmul on PE)
The host normalizes/casts/transposes the inputs (O(B*D) prep), then takes
logs of the gathered row/column sums and the exact diagonal to form the
scalar.  No max-subtraction is needed: |logits| <= 1/TAU = 10, so exp is
comfortably inside fp32 range.

Device layout per core:
  vT  [768, 1024] bf16  (normalized v slab, feature-major)
  uT  [768, 8192] bf16  (normalized u, feature-major)
  rs  [128, 8]    f32   row sums of exp:   row m*128+p  ->  rs[p, m]
  cs  [1, 8192]   f32   partial column sums over the 1024 local rows
"""

import sys

for _p in ("/opt/trn_rl_repo", "/root/.axon_site/_ro/trn_rl_repo"):
    if _p not in sys.path:
        sys.path.insert(0, _p)

import numpy as np
import ml_dtypes

TAU = 0.1
LAMBD = 0.75
EPS = 1e-8
B, D = 8192, 768
N_CORES = 8
M_ROWS = B // N_CORES          # 1024 rows of v per core
M_TILES = M_ROWS // 128        # 8
K_TILES = D // 128             # 6
NB = 8                         # column blocks of 1024
NB_W = B // NB                 # 1024 columns per block
NS = NB_W // 512               # 2 matmuls of N=512 per block
FP8_SCALE = 32.0               # host pre-scale before e4m3 cast

_CACHE = {}


def build_nc(repeat=1, for_sim=False, ablate=(), dtype_mode="fp8",
             cs_mode="dve", upool_bufs=3, epool_bufs=3, nb_w=None):
    """Build the per-core Bass module. `repeat` unrolls the whole pipeline
    that many times (for steady-state timing); outputs are overwritten each
    repetition so results stay correct.

    `ablate` (perf debugging only — wrong results): subset of
    {"nocs", "noexp", "nouT"} removing the column-sum matmuls, the exp
    activations, or the per-block uT DMA loads."""
    import concourse.mybir as mybir
    import concourse.tile as tile
    from concourse import bacc

    f32 = mybir.dt.float32
    bf16 = mybir.dt.bfloat16
    nbw = NB_W if nb_w is None else nb_w
    nb_count = B // nbw
    ns_count = nbw // 512
    wide = nbw > 1024          # S tiles use all 8 PSUM banks; cs borrows S slots
    in_dt = mybir.dt.float8e4 if dtype_mode == "fp8" else bf16
    # host pre-scales fp8 inputs by FP8_SCALE; undo inside the exp
    exp_scale = (1.0 / (TAU * FP8_SCALE * FP8_SCALE)
                 if dtype_mode == "fp8" else 1.0 / TAU)

    nc = bacc.Bacc("TRN2", target_bir_lowering=False)
    vT = nc.dram_tensor("vT", [D, M_ROWS], in_dt, kind="ExternalInput")
    uT = nc.dram_tensor("uT", [D, B], in_dt, kind="ExternalInput")
    rs_d = nc.dram_tensor("rs", [128, M_TILES], f32, kind="ExternalOutput")
    cs_d = nc.dram_tensor("cs", [1, B], f32, kind="ExternalOutput")

    with tile.TileContext(nc) as tc:
        with (
            tc.tile_pool(name="singles", bufs=1) as singles,
            tc.tile_pool(name="boundary", bufs=2) as boundary,
            tc.tile_pool(name="upool", bufs=upool_bufs) as upool,
            tc.tile_pool(name="epool", bufs=epool_bufs) as epool,
            tc.tile_pool(name="eaccpool", bufs=2) as eaccpool,
            tc.tile_pool(name="spool",
                         bufs=2 if wide else (3 if cs_mode == "dve" else 2),
                         space="PSUM") as spool,
            tc.tile_pool(name="cspool", bufs=2, space="PSUM") as cspool,
        ):
            ones = singles.tile([128, 1], bf16)
            nc.vector.memset(ones, 1.0)
            # Preload the exp table set while DMAs run.
            dummy = singles.tile([128, 1], f32)
            nc.vector.memset(dummy, 0.0)
            nc.scalar.activation(out=dummy, in_=dummy,
                                 func=mybir.ActivationFunctionType.Exp)

            vT_sb = singles.tile([128, K_TILES, M_ROWS], in_dt)
            nc.sync.dma_start(
                out=vT_sb[:, :, :],
                in_=vT.rearrange("(k p) b -> p k b", p=128))

            for rep in range(repeat):
                rs_parts = boundary.tile([128, M_TILES, nb_count], f32,
                                         tag="rs_parts")
                colsum_sb = boundary.tile([1, B], f32, tag="colsum_sb")

                for nb in range(nb_count):
                    uT_sb = upool.tile([128, K_TILES, nbw], in_dt, tag="uT")
                    if "nouT" not in ablate:
                        uT_src = uT.rearrange(
                            "(k p) b -> p k b", p=128)[
                            :, :, nb * nbw:(nb + 1) * nbw]
                        nc.sync.dma_start(out=uT_sb[:, :, :], in_=uT_src)

                    if cs_mode != "dve":
                        cs_ps = cspool.tile([1, nbw], f32, tag="cs")
                    e_acc = None
                    prev_E = None
                    for m in range(M_TILES):
                        s_ps = spool.tile([128, nbw], f32, tag="S")
                        if dtype_mode == "fp8":
                            for kp in range(K_TILES // 2):
                                lhsT = vT_sb[:, 2 * kp:2 * kp + 2,
                                             m * 128:(m + 1) * 128]
                                for ns in range(ns_count):
                                    nc.tensor.matmul(
                                        s_ps[:, ns * 512:(ns + 1) * 512],
                                        lhsT,
                                        uT_sb[:, 2 * kp:2 * kp + 2,
                                              ns * 512:(ns + 1) * 512],
                                        start=(kp == 0),
                                        stop=(kp == K_TILES // 2 - 1),
                                        perf_mode=mybir.MatmulPerfMode.DoubleRow,
                                    )
                        else:
                            for k in range(K_TILES):
                                lhsT = vT_sb[:, k, m * 128:(m + 1) * 128]
                                for ns in range(ns_count):
                                    nc.tensor.matmul(
                                        s_ps[:, ns * 512:(ns + 1) * 512],
                                        lhsT,
                                        uT_sb[:, k, ns * 512:(ns + 1) * 512],
                                        start=(k == 0),
                                        stop=(k == K_TILES - 1),
                                    )
                        # column-sum handling for the previous m's exp tile
                        # (delayed one iteration so PE never waits on ACT)
                        if prev_E is not None and "nocs" not in ablate:
                            if cs_mode == "dve":
                                if e_acc is None:
                                    e_acc = eaccpool.tile(
                                        [128, nbw], bf16, tag="EA")
                                    nc.vector.tensor_copy(
                                        out=e_acc, in_=prev_E)
                                else:
                                    nc.vector.tensor_add(
                                        out=e_acc, in0=e_acc, in1=prev_E)
                            else:
                                for ns in range(NS):
                                    nc.tensor.matmul(
                                        cs_ps[0:1, ns * 512:(ns + 1) * 512],
                                        ones,
                                        prev_E[:, ns * 512:(ns + 1) * 512],
                                        start=(m == 1),
                                        stop=False,
                                    )
                        e_sb = epool.tile([128, nbw], bf16, tag="E")
                        if "noexp" not in ablate:
                            nc.scalar.activation(
                                out=e_sb, in_=s_ps,
                                func=mybir.ActivationFunctionType.Exp,
                                scale=exp_scale,
                                accum_out=rs_parts[:, m, nb:nb + 1],
                            )
                        else:
                            nc.vector.tensor_copy(
                                out=rs_parts[:, m, nb:nb + 1],
                                in_=s_ps[:, 0:1])
                        prev_E = e_sb
                    if "nocs" not in ablate:
                        if cs_mode == "dve":
                            nc.vector.tensor_add(
                                out=e_acc, in0=e_acc, in1=prev_E)
                            if wide:
                                cs_ps = spool.tile([1, nbw], f32, tag="S")
                                for ns in range(ns_count):
                                    nc.tensor.matmul(
                                        cs_ps[0:1, ns * 512:(ns + 1) * 512],
                                        ones,
                                        e_acc[:, ns * 512:(ns + 1) * 512],
                                        start=True,
                                        stop=True,
                                    )
                                nc.vector.tensor_copy(
                                    out=colsum_sb[0:1,
                                                  nb * nbw:(nb + 1) * nbw],
                                    in_=cs_ps)
                            else:
                                for ns in range(ns_count):
                                    cs_ps = cspool.tile([1, 512], f32,
                                                        tag="cs")
                                    nc.tensor.matmul(
                                        cs_ps,
                                        ones,
                                        e_acc[:, ns * 512:(ns + 1) * 512],
                                        start=True,
                                        stop=True,
                                    )
                                    nc.vector.tensor_copy(
                                        out=colsum_sb[
                                            0:1,
                                            nb * nbw + ns * 512:
                                            nb * nbw + (ns + 1) * 512],
                                        in_=cs_ps)
                        else:
                            for ns in range(ns_count):
                                nc.tensor.matmul(
                                    cs_ps[0:1, ns * 512:(ns + 1) * 512],
                                    ones,
                                    prev_E[:, ns * 512:(ns + 1) * 512],
                                    start=False,
                                    stop=True,
                                )
                            nc.scalar.copy(
                                out=colsum_sb[0:1,
                                              nb * nbw:(nb + 1) * nbw],
                                in_=cs_ps)
                    else:
                        nc.vector.memset(colsum_sb[0:1, nb * nbw:nb * nbw + 1], 0.0)

                rs_fin = boundary.tile([128, M_TILES, 1], f32, tag="rs_fin")
                nc.vector.reduce_sum(out=rs_fin, in_=rs_parts,
                                     axis=mybir.AxisListType.X)
                nc.sync.dma_start(out=rs_d[:, :], in_=rs_fin[:, :, 0])
                nc.sync.dma_start(out=cs_d[:, :], in_=colsum_sb[:, :])

    if for_sim:
        nc.compile()
    else:
        nc.finalize()
    return nc


def prep_inputs(v, u, dtype_mode="fp8"):
    """Host-side prep: normalize rows, cast to the device dtype, transpose
    to feature-major, shard v across cores. Returns (in_maps, vn, un)."""
    v = np.asarray(v, dtype=np.float32)
    u = np.asarray(u, dtype=np.float32)
    vn = v / np.maximum(np.sqrt((v.astype(np.float64) ** 2).sum(1)),
                        EPS).astype(np.float32)[:, None]
    un = u / np.maximum(np.sqrt((u.astype(np.float64) ** 2).sum(1)),
                        EPS).astype(np.float32)[:, None]
    if dtype_mode == "fp8":
        dt = ml_dtypes.float8_e4m3
        vnT = np.ascontiguousarray((vn.T * FP8_SCALE).astype(dt))
        unT = np.ascontiguousarray((un.T * FP8_SCALE).astype(dt))
    else:
        vnT = np.ascontiguousarray(vn.T.astype(ml_dtypes.bfloat16))
        unT = np.ascontiguousarray(un.T.astype(ml_dtypes.bfloat16))
    in_maps = [
        {"vT": np.ascontiguousarray(vnT[:, c * M_ROWS:(c + 1) * M_ROWS]),
         "uT": unT}
        for c in range(N_CORES)
    ]
    return in_maps, vn, un


def combine(results, vn, un):
    """Host-side unshard: logs + exact diagonal + weighted mean."""
    rowsum = np.concatenate(
        [np.asarray(r["rs"], np.float64).T.reshape(-1) for r in results])
    colsum = np.sum(
        [np.asarray(r["cs"], np.float64)[0] for r in results], axis=0)
    diag = (vn.astype(np.float64) * un.astype(np.float64)).sum(1) / TAU
    lse_r = np.log(rowsum)
    lse_c = np.log(colsum)
    loss = np.mean(LAMBD * (lse_r - diag) + (1.0 - LAMBD) * (lse_c - diag))
    return np.asarray(loss, dtype=np.float32)


DTYPE_MODE = "fp8"


def kernel(v, u):
    from concourse.bass_utils import run_bass_kernel_spmd

    if "nc" not in _CACHE:
        _CACHE["nc"] = build_nc(dtype_mode=DTYPE_MODE)
    nc = _CACHE["nc"]
    in_maps, vn, un = prep_inputs(v, u, dtype_mode=DTYPE_MODE)
    res = run_bass_kernel_spmd(nc, in_maps, core_ids=list(range(N_CORES)))
    return combine(res.results, vn, un)


if __name__ == "__main__":
    rng = np.random.default_rng(0)
    v = rng.standard_normal((B, D), dtype=np.float32)
    u = rng.standard_normal((B, D), dtype=np.float32)
    out = kernel(v, u)
    print("kernel out:", out)



# revision 3
# speedup vs baseline: 1.0411x; 1.0411x over previous
"""ConVIRT loss kernel for 8 Trainium2 NeuronCores.

Computation (reference):
    vn = v / max(||v||, eps);  un = u / max(||u||, eps)          [8192, 768]
    sim = vn @ un.T / TAU                                        [8192, 8192]
    loss_it = logsumexp(sim, axis=1) - diag(sim)
    loss_ti = logsumexp(sim, axis=0) - diag(sim)
    out = mean(0.75 * loss_it + 0.25 * loss_ti)                  scalar

Sharding: rows of v are split across the 8 cores (1024 rows each); every
core holds all of u.  Core c computes its [1024, 8192] slab of
exp(sim / TAU) on the fly and reduces it two ways (row sums and column
sums); the host takes logs, adds the exact diagonal, and averages.

Per-core pipeline (v2), all in fp8:
  - PE: S = vT.T @ uT in fp8 DoubleRow ([128,512] tiles, K=768 via 3
    instructions), plus column sums via an fp8 DoubleRow ones-matmul on
    E pairs (ones[128,2,32] @ E[128,2,1024] -> [32,1024] PSUM,
    accumulated over the 4 m-pairs of each 1024-column block).
  - ACT (~2/3 of tiles): E = exp(S * es) -> fp8e4 SBUF with the row sum
    taken for free via the fp32 accumulator (accum_out).
  - DVE (~1/3 of tiles): fast-exp via exponent-bit arithmetic — the
    fp8e4 BITS of exp(x) are an affine function of x per binade:
    i8 = rint(A8*S + B8) computed by one tensor_scalar (f32 PSUM in,
    int8 out reinterpreted as fp8e4), with B8 calibrated so the
    piecewise-linear approximation is mean-unbiased (rel err ~3% RMS
    per element, ~0.1% per 1024-sum; loss impact ~1e-4).  A second
    in-place tensor_scalar supplies the row sum via accum_out.
    DVE also copies the per-block column sums PSUM -> SBUF.
Row/column sums use sums of the same fp8-quantized E on the colsum path
for both engines, so the two paths mix freely.  The host normalizes /
casts / transposes inputs, computes the exact diagonal, and takes logs.
No max-subtraction is needed: |logits| <= 1/TAU = 10.

Device layout per core:
  vT  [768, 1024] fp8  (normalized v slab * 32, feature-major)
  uT  [768, 8192] fp8  (normalized u * 32, feature-major)
  rs  [128, 8]    f32  row sums of exp:  row m*128+p -> rs[p, m]
  cs  [1, 8192]   f32  column sums over the 1024 local rows
"""

import sys

for _p in ("/opt/trn_rl_repo", "/root/.axon_site/_ro/trn_rl_repo"):
    if _p not in sys.path:
        sys.path.insert(0, _p)

import numpy as np
import ml_dtypes

TAU = 0.1
LAMBD = 0.75
EPS = 1e-8
B, D = 8192, 768
N_CORES = 8
M_ROWS = B // N_CORES          # 1024 rows of v per core
M_TILES = M_ROWS // 128        # 8
K_TILES = D // 128             # 6
NB = 8                         # column blocks of 1024
NB_W = B // NB                 # 1024 columns per block
FP8_SCALE = 32.0               # host pre-scale before e4m3 cast
ES = 1.0 / (TAU * FP8_SCALE * FP8_SCALE)   # exp arg = S * ES
# fast-exp constants: fp8e4 bits of exp(S*ES) ~= rint(A8*S + B8)
A8 = 8.0 * np.log2(np.e) * ES
B8 = 56.0 - 0.4560             # calibrated: mean-unbiased vs exact exp

_CACHE = {}


def _is_dve_unit(u, dve_mod=3, dve_off=1):
    return (u % dve_mod) == dve_off


def build_nc(repeat=1, for_sim=False, dtype_mode="fp8", dve_mod=3, dve_off=1,
             upool_bufs=2, epool_bufs=3, spool_bufs=3):
    """Per-core Bass module. `repeat` unrolls the pass for steady-state
    timing (outputs overwritten each repetition)."""
    import concourse.mybir as mybir
    import concourse.tile as tile
    from concourse import bacc

    f32 = mybir.dt.float32
    i8 = mybir.dt.int8
    fp8 = mybir.dt.float8e4
    DR = mybir.MatmulPerfMode.DoubleRow

    nc = bacc.Bacc("TRN2", target_bir_lowering=False)
    vT = nc.dram_tensor("vT", [D, M_ROWS], fp8, kind="ExternalInput")
    uT = nc.dram_tensor("uT", [D, B], fp8, kind="ExternalInput")
    rs_d = nc.dram_tensor("rs", [128, M_TILES], f32, kind="ExternalOutput")
    cs_d = nc.dram_tensor("cs", [1, B], f32, kind="ExternalOutput")

    with tile.TileContext(nc) as tc:
        with (
            tc.tile_pool(name="singles", bufs=1) as singles,
            tc.tile_pool(name="boundary", bufs=2) as boundary,
            tc.tile_pool(name="upool", bufs=upool_bufs) as upool,
            tc.tile_pool(name="epool", bufs=epool_bufs) as epool,
            tc.tile_pool(name="spool", bufs=spool_bufs, space="PSUM") as spool,
            tc.tile_pool(name="cspool", bufs=1, space="PSUM") as cspool,
        ):
            ones = singles.tile([128, 2, 32], fp8)
            nc.vector.memset(ones, 1.0)
            # Preload the exp table set while DMAs run.
            dummy = singles.tile([128, 1], f32)
            nc.vector.memset(dummy, 0.0)
            nc.scalar.activation(out=dummy, in_=dummy,
                                 func=mybir.ActivationFunctionType.Exp)

            vT_sb = singles.tile([128, K_TILES, M_ROWS], fp8)
            nc.sync.dma_start(
                out=vT_sb[:, :, :],
                in_=vT.rearrange("(k p) b -> p k b", p=128))

            for rep in range(repeat):
                rs_parts = boundary.tile([128, M_TILES, NB], f32,
                                         tag="rs_parts")
                colsum_sb = boundary.tile([1, B], f32, tag="colsum_sb")

                uT_sb = upool.tile([128, K_TILES, B], fp8, tag="uT")
                uT_r = uT.rearrange("(k p) b -> p k b", p=128)
                for nb in range(NB):
                    nc.sync.dma_start(
                        out=uT_sb[:, :, nb * NB_W:(nb + 1) * NB_W],
                        in_=uT_r[:, :, nb * NB_W:(nb + 1) * NB_W])

                pending_cs = []
                for nb in range(NB):
                    cs_ps = cspool.tile([32, NB_W], f32, tag="cs")
                    for m in range(M_TILES):
                        if m % 2 == 0:
                            ep = epool.tile([128, 2, NB_W], fp8, tag="E")
                        s = spool.tile([128, NB_W], f32, tag="S")
                        for kp in range(K_TILES // 2):
                            lhsT = vT_sb[:, 2 * kp:2 * kp + 2,
                                         m * 128:(m + 1) * 128]
                            for ns in range(NB_W // 512):
                                nc.tensor.matmul(
                                    s[:, ns * 512:(ns + 1) * 512],
                                    lhsT,
                                    uT_sb[:, 2 * kp:2 * kp + 2,
                                          nb * NB_W + ns * 512:
                                          nb * NB_W + (ns + 1) * 512],
                                    start=(kp == 0),
                                    stop=(kp == K_TILES // 2 - 1),
                                    perf_mode=DR,
                                )
                        # flush a delayed colsum matmul so PE never waits
                        # on the exp of the pair it reduces
                        for fn in pending_cs:
                            fn()
                        pending_cs = []
                        eh = ep[:, m % 2, :]
                        if _is_dve_unit(nb * M_TILES + m, dve_mod, dve_off):
                            nc.vector.tensor_scalar(
                                out=eh.bitcast(i8), in0=s,
                                scalar1=A8, scalar2=B8,
                                op0=mybir.AluOpType.mult,
                                op1=mybir.AluOpType.add)
                            nc.vector.tensor_scalar(
                                out=eh, in0=eh,
                                scalar1=1.0, scalar2=None,
                                op0=mybir.AluOpType.mult,
                                op1=mybir.AluOpType.add,
                                accum_out=rs_parts[:, m, nb:nb + 1])
                        else:
                            nc.scalar.activation(
                                out=eh, in_=s,
                                func=mybir.ActivationFunctionType.Exp,
                                scale=ES,
                                accum_out=rs_parts[:, m, nb:nb + 1])
                        if m % 2 == 1:
                            def make_cs(ep=ep, cs_ps=cs_ps, m=m):
                                def emit():
                                    for h in range(NB_W // 512):
                                        nc.tensor.matmul(
                                            cs_ps[:, h * 512:(h + 1) * 512],
                                            ones,
                                            ep[:, :, h * 512:(h + 1) * 512],
                                            start=(m == 1),
                                            stop=(m == M_TILES - 1),
                                            perf_mode=DR)
                                return emit
                            pending_cs.append(make_cs())
                    # copy this block's colsums out (row 0 of cs_ps).
                    # The last pair's matmul is still pending; emit the copy
                    # after it via a deferred closure too.
                    def make_copy(cs_ps=cs_ps, nb=nb):
                        def emit():
                            nc.vector.tensor_copy(
                                out=colsum_sb[0:1, nb * NB_W:(nb + 1) * NB_W],
                                in_=cs_ps[0:1, :])
                        return emit
                    pending_cs.append(make_copy())
                for fn in pending_cs:
                    fn()

                rs_fin = boundary.tile([128, M_TILES, 1], f32, tag="rs_fin")
                nc.vector.reduce_sum(out=rs_fin, in_=rs_parts,
                                     axis=mybir.AxisListType.X)
                nc.sync.dma_start(out=rs_d[:, :], in_=rs_fin[:, :, 0])
                nc.sync.dma_start(out=cs_d[:, :], in_=colsum_sb[:, :])

    if for_sim:
        nc.compile()
    else:
        nc.finalize()
    return nc


def prep_inputs(v, u, dtype_mode="fp8"):
    """Host-side prep: normalize rows, scale+cast to fp8e4, transpose to
    feature-major, shard v across cores. Returns (in_maps, vn, un)."""
    v = np.asarray(v, dtype=np.float32)
    u = np.asarray(u, dtype=np.float32)
    vn = v / np.maximum(np.sqrt((v.astype(np.float64) ** 2).sum(1)),
                        EPS).astype(np.float32)[:, None]
    un = u / np.maximum(np.sqrt((u.astype(np.float64) ** 2).sum(1)),
                        EPS).astype(np.float32)[:, None]
    dt = ml_dtypes.float8_e4m3
    vnT = np.ascontiguousarray((vn.T * FP8_SCALE).astype(dt))
    unT = np.ascontiguousarray((un.T * FP8_SCALE).astype(dt))
    in_maps = [
        {"vT": np.ascontiguousarray(vnT[:, c * M_ROWS:(c + 1) * M_ROWS]),
         "uT": unT}
        for c in range(N_CORES)
    ]
    return in_maps, vn, un


def combine(results, vn, un):
    """Host-side unshard: logs + exact diagonal + weighted mean."""
    rowsum = np.concatenate(
        [np.asarray(r["rs"], np.float64).T.reshape(-1) for r in results])
    colsum = np.sum(
        [np.asarray(r["cs"], np.float64)[0] for r in results], axis=0)
    diag = (vn.astype(np.float64) * un.astype(np.float64)).sum(1) / TAU
    lse_r = np.log(rowsum)
    lse_c = np.log(colsum)
    loss = np.mean(LAMBD * (lse_r - diag) + (1.0 - LAMBD) * (lse_c - diag))
    return np.asarray(loss, dtype=np.float32)


DTYPE_MODE = "fp8"


def kernel(v, u):
    from concourse.bass_utils import run_bass_kernel_spmd

    if "nc" not in _CACHE:
        _CACHE["nc"] = build_nc(dtype_mode=DTYPE_MODE)
    nc = _CACHE["nc"]
    in_maps, vn, un = prep_inputs(v, u, dtype_mode=DTYPE_MODE)
    res = run_bass_kernel_spmd(nc, in_maps, core_ids=list(range(N_CORES)))
    return combine(res.results, vn, un)


if __name__ == "__main__":
    rng = np.random.default_rng(0)
    v = rng.standard_normal((B, D), dtype=np.float32)
    u = rng.standard_normal((B, D), dtype=np.float32)
    out = kernel(v, u)
    print("kernel out:", out)
